# revision 29
# baseline (speedup 1.0000x reference)
"""Trainium2 Bass kernel for binarized 3x3 conv + batch-norm (BinConv2d).

Reference computation:
    xb = sign(x); wb = sign(weight)
    y  = conv2d(xb, wb, stride 1, pad 1)        # NCHW / OIHW
    out = batchnorm(y, batch stats over (N,H,W), affine gamma/beta)

Strategy: data-parallel over batch (64 images -> 8 images per NeuronCore).
The conv runs as shifted matmuls with Cin=128 on the SBUF partition dim,
accumulating in PSUM. Signs are cast to fp8 (e4m3, +/-1 exact) and the 3x3
taps are processed as 4 DoubleRow pairs + 1 single matmul per output tile
(~1.8x TensorE throughput vs bf16). Matmul tiles span 8 rows x 58 cols of
the zero-padded image so every tap's moving operand is one contiguous
464-element run; the two junk columns per row are skipped downstream.
Conv outputs are integers |y| <= 1152: exact in fp32 PSUM and in the fp16
SBUF copy. Channel stats come from DVE bn_stats/bn_aggr, are AllReduced
across the 8 cores, and the affine is applied on-device before the f32
output DMA.
"""
import numpy as np

import concourse.bacc as bacc
import concourse.tile as tile
import concourse.mybir as mybir
import concourse.bass_utils as bass_utils
from concourse.bass_types import AP

F32 = mybir.dt.float32
F16 = mybir.dt.float16
F8 = mybir.dt.float8e4
AF = mybir.ActivationFunctionType
ALU = mybir.AluOpType
DR = mybir.MatmulPerfMode.DoubleRow

N_CORES = 8
N_FULL = 64            # total batch
NIMG = N_FULL // N_CORES   # images per core
C = 128                # channels (in == out)
H = W = 56
WP = W + 2             # padded width (58)
HPHYS = H + 4          # physical rows: guard + pad + 56 + pad + guard
PSTRIDE = HPHYS * WP   # per-partition elements of one image tile
NT = 7                 # row tiles per image
RT = H // NT           # rows per tile (8)
TW = RT * WP           # moving free size per tile (464)
IMG = H * W            # 3136
COUNT = N_FULL * IMG   # global reduction count per channel
EPS = 1e-5

TRACE = False          # test.py may flip this to get an NTFF profile

_CACHE = {}


def _build(use_collective=True, nimg=NIMG):
    nc = bacc.Bacc("TRN2", target_bir_lowering=False, debug=False,
                   num_devices=N_CORES)
    x = nc.dram_tensor("x", [NIMG, C, H, W], F32, kind="ExternalInput").ap()
    wt = nc.dram_tensor("wt", [C, 9, C], F32, kind="ExternalInput").ap()
    gb = nc.dram_tensor("gb", [C, 2], F32, kind="ExternalInput").ap()
    out = nc.dram_tensor("out", [NIMG, C, H, W], F32, kind="ExternalOutput").ap()

    with tile.TileContext(nc) as tc:
        with tc.tile_pool(name="const", bufs=1) as pc, \
             tc.tile_pool(name="xstage", bufs=4) as pxs, \
             tc.tile_pool(name="xpad", bufs=3) as pxp, \
             tc.tile_pool(name="ostage", bufs=4) as pos, \
             tc.tile_pool(name="psum", bufs=8, space="PSUM") as pp, \
             tc.tile_pool(name="dram", bufs=1, space="DRAM") as pd:

            # ---- persistent buffers ----
            y16 = pc.tile([C, NIMG, H, W], F16)       # conv ints (exact)
            bnbuf = pc.tile([C, nimg * NT, 6], F32)
            epst = pc.tile([C, 1], F32)
            nc.vector.memset(epst[:], EPS)

            # ---- phase 1: conv + local stats, per image ----
            HH = H // 2
            wstage = pc.tile([C, 9, C], F32)
            wb = pc.tile([C, 9, C], F8)
            gbt = pc.tile([C, 2], F32)
            for n in range(nimg):
                # physical rows: 0 guard, 1 top pad, 2..57 image, 58 bottom
                # pad, 59 guard. Guards keep the deliberate 2-junk-column
                # overreads of the 58-wide matmul tiles inside the tile.
                xp = pxp.tile([C, HPHYS, WP], F8)
                nc.gpsimd.memset(xp[:, 0:2, :], 0.0)
                nc.gpsimd.memset(xp[:, HPHYS - 2:HPHYS, :], 0.0)
                nc.gpsimd.memset(xp[:, 2:HPHYS - 2, 0], 0.0)
                nc.gpsimd.memset(xp[:, 2:HPHYS - 2, WP - 1], 0.0)
                # DMA + sign in half-image chunks so matmuls start sooner
                for h in (0, HH):
                    xs = pxs.tile([C, HH, W], F32, tag="xs", name="xs")
                    nc.sync.dma_start(out=xs[:], in_=x[n, :, h:h + HH, :])
                    nc.scalar.activation(
                        out=xp[:, 2 + h:2 + h + HH, 1:WP - 1],
                        in_=xs[:], func=AF.Sign)

                if n == 0:
                    # weights: emitted after image 0's input chain so its
                    # DMA+sign stay at the head of every queue, but before
                    # the first matmul which reads wb
                    nc.sync.dma_start(out=wstage[:], in_=wt[:])
                    nc.scalar.activation(out=wb[:], in_=wstage[:],
                                         func=AF.Sign)
                    nc.sync.dma_start(out=gbt[:], in_=gb[:])

                psums = [pp.tile([C, TW], F32, tag="ps", name="ps")
                         for _ in range(NT)]

                def tap_off(h0, it):
                    dh, dw = it // 3 - 1, it % 3 - 1
                    return (h0 + 2 + dh) * WP + dw

                # tap-step outer, tile inner: consecutive matmuls share the
                # stationary operand
                for p in range(5):
                    for t in range(NT):
                        h0 = t * RT
                        if p < 4:
                            o0 = tap_off(h0, 2 * p)
                            o1 = tap_off(h0, 2 * p + 1)
                            rhs = AP(xp.tensor, xp.offset + o0,
                                     [[PSTRIDE, C], [o1 - o0, 2], [1, TW]])
                            nc.tensor.matmul(out=psums[t][:],
                                             lhsT=wb[:, 2 * p:2 * p + 2, :],
                                             rhs=rhs, start=(p == 0),
                                             stop=False, perf_mode=DR)
                        else:
                            o8 = tap_off(h0, 8)
                            rhs8 = AP(xp.tensor, xp.offset + o8,
                                      [[PSTRIDE, C], [1, TW]])
                            nc.tensor.matmul(out=psums[t][:], lhsT=wb[:, 8, :],
                                             rhs=rhs8, start=False, stop=True)

                for t in range(NT):
                    idx = n * NT + t
                    ps3 = psums[t][:].rearrange("p (r c) -> p r c", r=RT)
                    ydst = y16[:, n, t * RT:(t + 1) * RT, :]
                    # PSUM -> fp16 copy of the valid columns, alternating
                    # engines to balance ACT vs DVE load
                    if t % 2 == 0:
                        nc.scalar.copy(out=ydst, in_=ps3[:, :, 1:W + 1])
                    else:
                        nc.vector.tensor_copy(out=ydst, in_=ps3[:, :, 1:W + 1])
                    # DVE: count/mean/M2 from the contiguous fp16 copy
                    nc.vector.bn_stats(
                        out=bnbuf[:, idx, :],
                        in_=ydst.rearrange("p r c -> p (r c)"))

                if n == 1 and use_collective:
                    # warm up the collectives firmware mid-conv (off the
                    # startup critical path) so the real AllGather's trigger
                    # latency is short
                    wbin = pd.tile([C, 1], F32)
                    wbout = pd.tile([C, 1], F32)
                    nc.sync.dma_start(out=wbin[:], in_=epst[:])
                    nc.gpsimd.collective_compute(
                        "AllReduce", ALU.add,
                        replica_groups=[list(range(N_CORES))],
                        ins=[wbin.opt()], outs=[wbout.opt()])

            # ---- phase 2: local bn_aggr, AllGather [mean,var], merge ----
            mv = pc.tile([C, 2], F32)
            mvl = pc.tile([C, 2], F32)
            nc.vector.bn_aggr(out=mvl[:],
                              in_=bnbuf[:].rearrange("p a s -> p (a s)"))
            if use_collective:
                bag_in = pd.tile([C, 2], F32)
                bag_out = pd.tile([N_CORES * C, 2], F32, addr_space="Shared")
                nc.sync.dma_start(out=bag_in[:], in_=mvl[:])
                nc.gpsimd.collective_compute(
                    "AllGather", ALU.bypass,
                    replica_groups=[list(range(N_CORES))],
                    ins=[bag_in.opt()], outs=[bag_out.opt()])
                gmv = pc.tile([C, N_CORES, 2], F32)
                src = AP(bag_out.tensor, bag_out.offset,
                         [[2, C], [C * 2, N_CORES], [1, 2]])
                nc.sync.dma_start(out=gmv[:], in_=src)
                # equal-count merge: meanG = avg(means);
                # varG = avg(vars) + avg(means^2) - meanG^2
                e2 = pc.tile([C, N_CORES], F32)
                nc.vector.tensor_mul(e2[:], gmv[:, :, 0], gmv[:, :, 0])
                nc.vector.tensor_add(e2[:], e2[:], gmv[:, :, 1])
                nc.vector.tensor_reduce(out=mv[:, 0:1], in_=gmv[:, :, 0],
                                        axis=mybir.AxisListType.X, op=ALU.add)
                nc.vector.tensor_reduce(out=mv[:, 1:2], in_=e2[:],
                                        axis=mybir.AxisListType.X, op=ALU.add)
                nc.vector.tensor_scalar_mul(mv[:], mv[:], 1.0 / N_CORES)
                msq = pc.tile([C, 1], F32)
                nc.vector.tensor_mul(msq[:], mv[:, 0:1], mv[:, 0:1])
                nc.vector.tensor_sub(mv[:, 1:2], mv[:, 1:2], msq[:])
            else:
                nc.vector.tensor_copy(out=mv[:], in_=mvl[:])

            # scale = gamma / sqrt(var + eps); bias = beta - mean * scale
            std_t = pc.tile([C, 1], F32)
            inv_t = pc.tile([C, 1], F32)
            scale_t = pc.tile([C, 1], F32)
            bias_t = pc.tile([C, 1], F32)
            tmp_t = pc.tile([C, 1], F32)
            nc.scalar.activation(out=std_t[:], in_=mv[:, 1:2], func=AF.Sqrt,
                                 bias=epst[:])
            nc.vector.reciprocal(inv_t[:], std_t[:])
            nc.vector.tensor_mul(scale_t[:], gbt[:, 0:1], inv_t[:])
            nc.vector.tensor_mul(tmp_t[:], mv[:, 0:1], scale_t[:])
            nc.vector.tensor_sub(bias_t[:], gbt[:, 1:2], tmp_t[:])

            # ---- phase 3: affine + store, half-image chunks on ACT+DVE ----
            for n in range(nimg):
                for ci, h in enumerate((0, HH)):
                    ot = pos.tile([C, HH, W], F32, tag="ot", name="ot")
                    ysrc = y16[:, n, h:h + HH, :]
                    if (2 * n + ci) % 2 == 0:
                        nc.vector.tensor_scalar(
                            ot[:], ysrc, scale_t[:, 0:1], bias_t[:, 0:1],
                            ALU.mult, ALU.add)
                    else:
                        nc.scalar.activation(
                            out=ot[:], in_=ysrc, func=AF.Identity,
                            bias=bias_t[:, 0:1], scale=scale_t[:, 0:1])
                    nc.sync.dma_start(out=out[n, :, h:h + HH, :], in_=ot[:])

    nc.compile()
    return nc


def kernel(x, weight, gamma, beta):
    x = np.asarray(x, dtype=np.float32)
    weight = np.asarray(weight, dtype=np.float32)
    gamma = np.asarray(gamma, dtype=np.float32)
    beta = np.asarray(beta, dtype=np.float32)

    if "nc" not in _CACHE:
        _CACHE["nc"] = _build()
    nc = _CACHE["nc"]

    # wt[ci, kh*3+kw, co] = weight[co, ci, kh, kw]
    wt = np.ascontiguousarray(weight.transpose(1, 2, 3, 0)).reshape(C, 9, C)
    gb = np.ascontiguousarray(np.stack([gamma, beta], axis=1))

    in_maps = []
    for i in range(N_CORES):
        in_maps.append({
            "x": np.ascontiguousarray(x[i * NIMG:(i + 1) * NIMG]),
            "wt": wt,
            "gb": gb,
        })

    res = bass_utils.run_bass_kernel_spmd(
        nc, in_maps, core_ids=list(range(N_CORES)), trace=TRACE)
    _CACHE["last_result"] = res

    out = np.empty((N_FULL, C, H, W), dtype=np.float32)
    for i in range(N_CORES):
        out[i * NIMG:(i + 1) * NIMG] = res.results[i]["out"]
    return out


# revision 32
# speedup vs baseline: 1.0272x; 1.0272x over previous
"""Trainium2 Bass kernel for binarized 3x3 conv + batch-norm (BinConv2d).

Reference computation:
    xb = sign(x); wb = sign(weight)
    y  = conv2d(xb, wb, stride 1, pad 1)        # NCHW / OIHW
    out = batchnorm(y, batch stats over (N,H,W), affine gamma/beta)

Strategy: data-parallel over batch (64 images -> 8 images per NeuronCore).
The conv runs as shifted matmuls with Cin=128 on the SBUF partition dim,
accumulating in PSUM. Signs are cast to fp8 (e4m3, +/-1 exact) and the 3x3
taps are processed as 4 DoubleRow pairs + 1 single matmul per output tile
(~1.8x TensorE throughput vs bf16). Matmul tiles span 8 rows x 58 cols of
the zero-padded image so every tap's moving operand is one contiguous
464-element run; the two junk columns per row are skipped downstream.
Conv outputs are integers |y| <= 1152: exact in fp32 PSUM and in the fp16
SBUF copy. Channel stats come from DVE bn_stats/bn_aggr, are AllReduced
across the 8 cores, and the affine is applied on-device before the f32
output DMA.
"""
import numpy as np

import concourse.bacc as bacc
import concourse.tile as tile
import concourse.mybir as mybir
import concourse.bass_utils as bass_utils
from concourse.bass_types import AP

F32 = mybir.dt.float32
F16 = mybir.dt.float16
F8 = mybir.dt.float8e4
AF = mybir.ActivationFunctionType
ALU = mybir.AluOpType
DR = mybir.MatmulPerfMode.DoubleRow

N_CORES = 8
N_FULL = 64            # total batch
NIMG = N_FULL // N_CORES   # images per core
C = 128                # channels (in == out)
H = W = 56
WP = W + 2             # padded width (58)
HPHYS = H + 4          # physical rows: guard + pad + 56 + pad + guard
PSTRIDE = HPHYS * WP   # per-partition elements of one image tile
NT = 7                 # row tiles per image
RT = H // NT           # rows per tile (8)
TW = RT * WP           # moving free size per tile (464)
IMG = H * W            # 3136
COUNT = N_FULL * IMG   # global reduction count per channel
EPS = 1e-5

TRACE = False          # test.py may flip this to get an NTFF profile

_CACHE = {}


def _build(use_collective=True, nimg=NIMG):
    nc = bacc.Bacc("TRN2", target_bir_lowering=False, debug=False,
                   num_devices=N_CORES)
    x = nc.dram_tensor("x", [NIMG, C, H, W], F32, kind="ExternalInput").ap()
    wt = nc.dram_tensor("wt", [C, 9, C], F32, kind="ExternalInput").ap()
    gb = nc.dram_tensor("gb", [C, 2], F32, kind="ExternalInput").ap()
    out = nc.dram_tensor("out", [NIMG, C, H, W], F32, kind="ExternalOutput").ap()

    with tile.TileContext(nc) as tc:
        with tc.tile_pool(name="const", bufs=1) as pc, \
             tc.tile_pool(name="xstage", bufs=4) as pxs, \
             tc.tile_pool(name="xpad", bufs=3) as pxp, \
             tc.tile_pool(name="ostage", bufs=4) as pos, \
             tc.tile_pool(name="psum", bufs=8, space="PSUM") as pp, \
             tc.tile_pool(name="dram", bufs=1, space="DRAM") as pd:

            # ---- persistent buffers ----
            y16 = pc.tile([C, NIMG, H, W], F16)       # conv ints (exact)
            bnbuf = pc.tile([C, nimg * NT, 6], F32)
            epst = pc.tile([C, 1], F32)
            nc.vector.memset(epst[:], EPS)

            # ---- phase 1: conv + local stats, per image ----
            HH = H // 2
            wstage = pc.tile([C, 9, C], F32)
            wb = pc.tile([C, 9, C], F8)
            gbt = pc.tile([C, 2], F32)
            for n in range(nimg):
                # physical rows: 0 guard, 1 top pad, 2..57 image, 58 bottom
                # pad, 59 guard. Guards keep the deliberate 2-junk-column
                # overreads of the 58-wide matmul tiles inside the tile.
                xp = pxp.tile([C, HPHYS, WP], F8)
                nc.gpsimd.memset(xp[:, 0:2, :], 0.0)
                nc.gpsimd.memset(xp[:, HPHYS - 2:HPHYS, :], 0.0)
                nc.gpsimd.memset(xp[:, 2:HPHYS - 2, 0], 0.0)
                nc.gpsimd.memset(xp[:, 2:HPHYS - 2, WP - 1], 0.0)
                if n == 0:
                    # weights first: the wsign must clear the ACT queue
                    # before image 0's signs so matmuls can start early
                    nc.sync.dma_start(out=wstage[:], in_=wt[:])
                    nc.scalar.activation(out=wb[:], in_=wstage[:],
                                         func=AF.Sign)
                # DMA + sign in half-image chunks so matmuls start sooner
                for ci, h in enumerate((0, HH)):
                    xs = pxs.tile([C, HH, W], F32, tag="xs", name="xs")
                    nc.sync.dma_start(out=xs[:], in_=x[n, :, h:h + HH, :])
                    xpdst = xp[:, 2 + h:2 + h + HH, 1:WP - 1]
                    if n == 0 and ci == 1:
                        # first image: sign the second half on DVE (2 passes,
                        # (x>=0)*2-1) in parallel with ACT signing the first
                        nc.vector.tensor_scalar(xpdst, xs[:], 0.0, 2.0,
                                                ALU.is_ge, ALU.mult)
                        nc.vector.tensor_scalar_add(xpdst, xpdst, -1.0)
                    else:
                        nc.scalar.activation(out=xpdst, in_=xs[:],
                                             func=AF.Sign)

                if n == 0:
                    nc.sync.dma_start(out=gbt[:], in_=gb[:])

                psums = [pp.tile([C, TW], F32, tag="ps", name="ps")
                         for _ in range(NT)]

                def tap_off(h0, it):
                    dh, dw = it // 3 - 1, it % 3 - 1
                    return (h0 + 2 + dh) * WP + dw

                # tap-step outer, tile inner: consecutive matmuls share the
                # stationary operand
                for p in range(5):
                    for t in range(NT):
                        h0 = t * RT
                        if p < 4:
                            o0 = tap_off(h0, 2 * p)
                            o1 = tap_off(h0, 2 * p + 1)
                            rhs = AP(xp.tensor, xp.offset + o0,
                                     [[PSTRIDE, C], [o1 - o0, 2], [1, TW]])
                            nc.tensor.matmul(out=psums[t][:],
                                             lhsT=wb[:, 2 * p:2 * p + 2, :],
                                             rhs=rhs, start=(p == 0),
                                             stop=False, perf_mode=DR)
                        else:
                            o8 = tap_off(h0, 8)
                            rhs8 = AP(xp.tensor, xp.offset + o8,
                                      [[PSTRIDE, C], [1, TW]])
                            nc.tensor.matmul(out=psums[t][:], lhsT=wb[:, 8, :],
                                             rhs=rhs8, start=False, stop=True)

                for t in range(NT):
                    idx = n * NT + t
                    ps3 = psums[t][:].rearrange("p (r c) -> p r c", r=RT)
                    ydst = y16[:, n, t * RT:(t + 1) * RT, :]
                    # PSUM -> fp16 copy of the valid columns, alternating
                    # engines to balance ACT vs DVE load
                    if t % 2 == 0:
                        nc.scalar.copy(out=ydst, in_=ps3[:, :, 1:W + 1])
                    else:
                        nc.vector.tensor_copy(out=ydst, in_=ps3[:, :, 1:W + 1])
                    # DVE: count/mean/M2 from the contiguous fp16 copy
                    nc.vector.bn_stats(
                        out=bnbuf[:, idx, :],
                        in_=ydst.rearrange("p r c -> p (r c)"))

                if n == 1 and use_collective:
                    # warm up the collectives firmware mid-conv (off the
                    # startup critical path) so the real AllGather's trigger
                    # latency is short
                    wbin = pd.tile([C, 1], F32)
                    wbout = pd.tile([C, 1], F32)
                    nc.sync.dma_start(out=wbin[:], in_=epst[:])
                    nc.gpsimd.collective_compute(
                        "AllReduce", ALU.add,
                        replica_groups=[list(range(N_CORES))],
                        ins=[wbin.opt()], outs=[wbout.opt()])

            # ---- phase 2: local bn_aggr, AllGather [mean,var], merge ----
            mv = pc.tile([C, 2], F32)
            mvl = pc.tile([C, 2], F32)
            nc.vector.bn_aggr(out=mvl[:],
                              in_=bnbuf[:].rearrange("p a s -> p (a s)"))
            if use_collective:
                bag_in = pd.tile([C, 2], F32)
                bag_out = pd.tile([N_CORES * C, 2], F32, addr_space="Shared")
                nc.sync.dma_start(out=bag_in[:], in_=mvl[:])
                nc.gpsimd.collective_compute(
                    "AllGather", ALU.bypass,
                    replica_groups=[list(range(N_CORES))],
                    ins=[bag_in.opt()], outs=[bag_out.opt()])
                gmv = pc.tile([C, N_CORES, 2], F32)
                src = AP(bag_out.tensor, bag_out.offset,
                         [[2, C], [C * 2, N_CORES], [1, 2]])
                nc.sync.dma_start(out=gmv[:], in_=src)
                # equal-count merge: meanG = avg(means);
                # varG = avg(vars) + avg(means^2) - meanG^2
                e2 = pc.tile([C, N_CORES], F32)
                nc.vector.tensor_mul(e2[:], gmv[:, :, 0], gmv[:, :, 0])
                nc.vector.tensor_add(e2[:], e2[:], gmv[:, :, 1])
                nc.vector.tensor_reduce(out=mv[:, 0:1], in_=gmv[:, :, 0],
                                        axis=mybir.AxisListType.X, op=ALU.add)
                nc.vector.tensor_reduce(out=mv[:, 1:2], in_=e2[:],
                                        axis=mybir.AxisListType.X, op=ALU.add)
                nc.vector.tensor_scalar_mul(mv[:], mv[:], 1.0 / N_CORES)
                msq = pc.tile([C, 1], F32)
                nc.vector.tensor_mul(msq[:], mv[:, 0:1], mv[:, 0:1])
                nc.vector.tensor_sub(mv[:, 1:2], mv[:, 1:2], msq[:])
            else:
                nc.vector.tensor_copy(out=mv[:], in_=mvl[:])

            # scale = gamma / sqrt(var + eps); bias = beta - mean * scale
            std_t = pc.tile([C, 1], F32)
            inv_t = pc.tile([C, 1], F32)
            scale_t = pc.tile([C, 1], F32)
            bias_t = pc.tile([C, 1], F32)
            tmp_t = pc.tile([C, 1], F32)
            nc.scalar.activation(out=std_t[:], in_=mv[:, 1:2], func=AF.Sqrt,
                                 bias=epst[:])
            nc.vector.reciprocal(inv_t[:], std_t[:])
            nc.vector.tensor_mul(scale_t[:], gbt[:, 0:1], inv_t[:])
            nc.vector.tensor_mul(tmp_t[:], mv[:, 0:1], scale_t[:])
            nc.vector.tensor_sub(bias_t[:], gbt[:, 1:2], tmp_t[:])

            # ---- phase 3: affine + store, half-image chunks on ACT+DVE ----
            for n in range(nimg):
                for ci, h in enumerate((0, HH)):
                    ot = pos.tile([C, HH, W], F32, tag="ot", name="ot")
                    ysrc = y16[:, n, h:h + HH, :]
                    if (2 * n + ci) % 2 == 0:
                        nc.vector.tensor_scalar(
                            ot[:], ysrc, scale_t[:, 0:1], bias_t[:, 0:1],
                            ALU.mult, ALU.add)
                    else:
                        nc.scalar.activation(
                            out=ot[:], in_=ysrc, func=AF.Identity,
                            bias=bias_t[:, 0:1], scale=scale_t[:, 0:1])
                    nc.sync.dma_start(out=out[n, :, h:h + HH, :], in_=ot[:])

    nc.compile()
    return nc


def kernel(x, weight, gamma, beta):
    x = np.asarray(x, dtype=np.float32)
    weight = np.asarray(weight, dtype=np.float32)
    gamma = np.asarray(gamma, dtype=np.float32)
    beta = np.asarray(beta, dtype=np.float32)

    if "nc" not in _CACHE:
        _CACHE["nc"] = _build()
    nc = _CACHE["nc"]

    # wt[ci, kh*3+kw, co] = weight[co, ci, kh, kw]
    wt = np.ascontiguousarray(weight.transpose(1, 2, 3, 0)).reshape(C, 9, C)
    gb = np.ascontiguousarray(np.stack([gamma, beta], axis=1))

    in_maps = []
    for i in range(N_CORES):
        in_maps.append({
            "x": np.ascontiguousarray(x[i * NIMG:(i + 1) * NIMG]),
            "wt": wt,
            "gb": gb,
        })

    res = bass_utils.run_bass_kernel_spmd(
        nc, in_maps, core_ids=list(range(N_CORES)), trace=TRACE)
    _CACHE["last_result"] = res

    out = np.empty((N_FULL, C, H, W), dtype=np.float32)
    for i in range(N_CORES):
        out[i * NIMG:(i + 1) * NIMG] = res.results[i]["out"]
    return out
